# revision 1
# baseline (speedup 1.0000x reference)
"""Multi-hot embedding bag kernel for Trainium2 (8 NeuronCores, batch-sharded).

Computes, for 5 feature groups g with multi-hot int32 matrices A_g [B, V_g]
and weights W_g [V_g, 64]:
    out = concat_g(norm_g(A_g @ W_g))  with the original module's quirks:
    - "decades" is normalized by its own row-sum AND by the movie row-sum
    - "movies" is never normalized
    - remaining groups are normalized by their own row-sum (rows with sum 0
      are left unnormalized)

Strategy per core (256 batch rows = 2 tiles of 128):
  - A slabs stream HBM->SBUF via gpsimd (SWDGE) DMA with int32->fp16 cast
  - each 128x128 chunk is transposed on the PE with a regular fp16 matmul
    against an identity (vocab must sit on partitions for the contraction)
  - transposed chunks land in PSUM, are copied 4-at-a-time (2 chunks x both
    batch tiles, alternating DVE/ACT) to SBUF as fp16
  - per chunk, ONE fp16 matmul with the host-packed [W_g | 1] chunk as the
    stationary operand and both batch tiles [128, 256] moving accumulates a
    transposed [65, 256] result in PSUM; row 64 is the row-sum
  - at group end the [65, 256] accumulator is copied to SBUF and transposed
    back on the PE (fp32 identity), then normalized with per-row reciprocals
"""

import math

import numpy as np

import concourse.bass as bass
import concourse.tile as tile
from concourse import bacc, mybir
from concourse.bass_utils import run_bass_kernel_spmd
from concourse.masks import make_identity

B = 2048
LF = 64
FE = LF + 1  # weights + ones column
N_CORES = 8
BPC = B // N_CORES  # 256 batch rows per core
P = 128
SLAB_CH = 32  # vocab chunks of 128 per A-slab DMA (32 -> 2 MiB int32 reads)

# (key, idx input name, weight input name, vocab size, output column offset)
# Processing order puts movies first so its row-sum reciprocal exists when
# decades is normalized.
GROUPS = [
    ("mov", "movie_idxs", "W_mov", 60000, 64),
    ("dec", "decade_idxs", "W_dec", 12, 0),
    ("cat", "category_idxs", "W_cat", 32, 128),
    ("per", "person_idxs", "W_per", 100000, 192),
    ("com", "company_idxs", "W_com", 20000, 256),
]
OUT_COLS = 5 * LF

_FP16 = mybir.dt.float16
_FP32 = mybir.dt.float32


def _build() -> bass.Bass:
    nc = bacc.Bacc(None, target_bir_lowering=False)

    a_dram = {}
    w_dram = {}
    for key, _, _, v, _ in GROUPS:
        c = math.ceil(v / P)
        a_dram[key] = nc.dram_tensor(f"a_{key}", [BPC, v], mybir.dt.int32,
                                     kind="ExternalInput")
        w_dram[key] = nc.dram_tensor(f"w_{key}", [P, c * FE], _FP16,
                                     kind="ExternalInput")
    out = nc.dram_tensor("out", [BPC, OUT_COLS], _FP32, kind="ExternalOutput")

    copy_flip = 0  # alternate PSUM->SBUF copies between DVE and ACT

    with tile.TileContext(nc) as tc:
        with (
            tc.tile_pool(name="singles", bufs=1) as singles,
            tc.tile_pool(name="apool", bufs=4) as apool,
            tc.tile_pool(name="wpool", bufs=4) as wpool,
            tc.tile_pool(name="atpool", bufs=4) as atpool,
            tc.tile_pool(name="npool", bufs=4) as npool,
            tc.tile_pool(name="ptp", bufs=2, space="PSUM") as ptp,
            tc.tile_pool(name="accp", bufs=2, space="PSUM") as accp,
            tc.tile_pool(name="backp", bufs=1, space="PSUM") as backp,
        ):
            ident16 = singles.tile([P, P], _FP16)
            make_identity(nc, ident16)
            ident32 = singles.tile([P, P], _FP32)
            make_identity(nc, ident32)

            out_sb = [singles.tile([P, OUT_COLS], _FP32, name=f"out_sb{i}")
                      for i in range(2)]
            rmov = [singles.tile([P, 1], _FP32, name=f"rmov{i}")
                    for i in range(2)]

            for key, _, _, v, col in GROUPS:
                n_ch = math.ceil(v / P)
                accT = accp.tile([FE, 2 * P], _FP32, tag="acc",
                                 name=f"accT_{key}")
                ch_done = 0
                for c0 in range(0, n_ch, SLAB_CH):
                    ch = min(SLAB_CH, n_ch - c0)
                    w_sb = wpool.tile([P, SLAB_CH, FE], _FP16, tag="w")
                    nc.sync.dma_start(
                        w_sb[:, :ch, :],
                        w_dram[key][:, c0 * FE:(c0 + ch) * FE].rearrange(
                            "p (c f) -> p c f", f=FE),
                    )
                    v0 = c0 * P
                    real_w = min(v, v0 + ch * P) - v0
                    a_sbs = []
                    for bt in range(2):
                        a_sb = apool.tile([P, SLAB_CH * P], _FP16, tag=f"a{bt}")
                        nc.gpsimd.dma_start(
                            a_sb[:, :real_w],
                            a_dram[key][bt * P:(bt + 1) * P, v0:v0 + real_w],
                        )
                        if real_w < ch * P:
                            nc.gpsimd.memset(a_sb[:, real_w:ch * P], 0.0)
                        a_sbs.append(a_sb)
                    for cb in range(0, ch, 4):
                        nb = min(4, ch - cb)
                        pt = ptp.tile([P, 8 * P], _FP32, tag="pt")
                        for j in range(nb):
                            for bt in range(2):
                                nc.tensor.matmul(
                                    pt[:, bass.ts(2 * j + bt, P)],
                                    lhsT=a_sbs[bt][:, bass.ts(cb + j, P)],
                                    rhs=ident16,
                                    start=True, stop=True,
                                )
                        at = atpool.tile([P, 4, 2 * P], _FP16, tag="at")
                        if copy_flip & 1:
                            nc.vector.tensor_copy(at[:, :nb, :],
                                                  pt[:, :nb * 2 * P])
                        else:
                            nc.scalar.copy(at[:, :nb, :], pt[:, :nb * 2 * P])
                        copy_flip += 1
                        for j in range(nb):
                            cidx = ch_done + cb + j
                            nc.tensor.matmul(
                                accT,
                                lhsT=w_sb[:, cb + j, :],
                                rhs=at[:, j, :],
                                start=(cidx == 0),
                                stop=(cidx == n_ch - 1),
                            )
                    ch_done += ch

                accT_sb = npool.tile([FE, 2 * P], _FP32, tag="accsb")
                nc.vector.tensor_copy(accT_sb, accT)
                for bt in range(2):
                    out2 = backp.tile([P, FE], _FP32, tag="out2")
                    nc.tensor.matmul(
                        out2,
                        lhsT=accT_sb[:, bass.ts(bt, P)],
                        rhs=ident32[:FE, :FE],
                        start=True, stop=True,
                    )
                    s = npool.tile([P, 1], _FP32, tag="s")
                    nc.vector.tensor_scalar_max(s, out2[:, LF:FE], 1.0)
                    nc.vector.reciprocal(s, s)
                    if key == "mov":
                        # movies are left unnormalized; stash 1/max(sum,1)
                        # for the decades double-normalization
                        nc.vector.tensor_copy(rmov[bt], s)
                        nc.scalar.copy(out_sb[bt][:, col:col + LF],
                                       out2[:, :LF])
                    else:
                        if key == "dec":
                            nc.vector.tensor_mul(s, s, rmov[bt])
                        nc.vector.tensor_scalar_mul(
                            out_sb[bt][:, col:col + LF], out2[:, :LF], s)

            for bt in range(2):
                nc.sync.dma_start(out[bt * P:(bt + 1) * P, :], out_sb[bt])

    nc.finalize()
    return nc


_NC_CACHE: bass.Bass | None = None


def _get_nc() -> bass.Bass:
    global _NC_CACHE
    if _NC_CACHE is None:
        _NC_CACHE = _build()
    return _NC_CACHE


def _pack_weights(w: np.ndarray) -> np.ndarray:
    """[V, 64] fp32 -> [128, C*65] fp16 with ones column, zero row padding,
    laid out so chunk c / partition p / feature f = row c*128+p of [W | 1]."""
    v = w.shape[0]
    c = math.ceil(v / P)
    we = np.concatenate([w.astype(np.float32),
                        np.ones((v, 1), np.float32)], axis=1).astype(np.float16)
    if c * P > v:
        we = np.concatenate([we, np.zeros((c * P - v, FE), np.float16)], axis=0)
    return np.ascontiguousarray(
        we.reshape(c, P, FE).transpose(1, 0, 2).reshape(P, c * FE))


def kernel(**inputs: np.ndarray) -> np.ndarray:
    import os

    nc = _get_nc()

    packed = {}
    for key, _, wname, _, _ in GROUPS:
        packed[f"w_{key}"] = _pack_weights(np.asarray(inputs[wname]))

    in_maps = []
    for core in range(N_CORES):
        m = dict(packed)
        sl = slice(core * BPC, (core + 1) * BPC)
        for key, aname, _, _, _ in GROUPS:
            m[f"a_{key}"] = np.ascontiguousarray(
                np.asarray(inputs[aname], dtype=np.int32)[sl])
        in_maps.append(m)

    trace = bool(int(os.environ.get("EMB_TRACE", "0")))
    res = run_bass_kernel_spmd(nc, in_maps, core_ids=list(range(N_CORES)),
                               trace=trace)
    if trace and res.exec_time_ns is not None:
        print(f"HW exec time: {res.exec_time_ns} ns")
        if res.instructions_and_trace is not None:
            print(f"trace: {res.instructions_and_trace[1]}")

    return np.concatenate([r["out"] for r in res.results], axis=0)



# revision 9
# speedup vs baseline: 3.0755x; 3.0755x over previous
"""Multi-hot embedding bag kernel for Trainium2 (8 NeuronCores, vocab-sharded).

Computes, for 5 feature groups g with multi-hot int32 matrices A_g [B, V_g]
and weights W_g [V_g, 64]:
    out = concat_g(norm_g(A_g @ W_g))  with the original module's quirks:
    - "decades" is normalized by its own row-sum AND by the movie row-sum
    - "movies" is never normalized
    - remaining groups are normalized by their own row-sum (rows with sum 0
      are left unnormalized)

Strategy (tensor-parallel over the vocab dim):
  - the tiny dec (V=12) and cat (V=32) groups are computed on the host
  - mov/per/com vocabs are split 8 ways; each core contracts its vocab
    slice against the full batch of 2048 rows
  - A is repacked on the host to fp8 ({0,1} exact, 1 byte -> 4x less HBM
    traffic than int32), pre-transposed to [vocab, batch] so vocab sits on
    partitions with no on-device transposes, and laid out chunk-major per
    partition so slab DMAs are fully contiguous 8 KiB-per-partition reads
  - slab DMAs round-robin over three DMA queues (sync + scalar HWDGE and
    the gpsimd SWDGE) - a single queue caps at ~230 GB/s of packet
    processing, well under the ~366 GB/s HBM port
  - per 128-row vocab chunk, the host-packed [W_g | 1] fp16 chunk is the
    stationary operand; 4 matmuls (one per 512-batch PSUM bank) accumulate
    [65, 2048] per group; row 64 is the multi-hot row-sum
  - partial [65, 2048] results stream back; the host sums partials across
    cores, applies the row-sum normalizations, and concatenates
"""

import math
import os

import ml_dtypes
import numpy as np

import concourse.bass as bass
import concourse.tile as tile
from concourse import bacc, mybir
from concourse.bass_utils import run_bass_kernel_spmd

B = 2048
LF = 64
FE = LF + 1  # weights + ones column
N_CORES = 8
P = 128
SLAB = 4  # vocab chunks per A slab DMA (8 KiB/partition)

# (key, idx input name, weight input name, vocab size, chunks-of-128 per core)
DEV_GROUPS = [
    ("mov", "movie_idxs", "W_mov", 60000, 59),
    ("per", "person_idxs", "W_per", 100000, 98),
    ("com", "company_idxs", "W_com", 20000, 20),
]
HOST_GROUPS = [
    ("dec", "decade_idxs", "W_dec", 12),
    ("cat", "category_idxs", "W_cat", 32),
]

_FP8 = mybir.dt.float8e4
_FP16 = mybir.dt.float16
_FP32 = mybir.dt.float32
_NP_FP8 = ml_dtypes.float8_e4m3
_FP8_ONE = 0x38  # bit pattern of 1.0 in e4m3 (bias 7)


def _build() -> bass.Bass:
    nc = bacc.Bacc(None, target_bir_lowering=False)

    at_dram = {}
    w_dram = {}
    for key, _, _, _, dc in DEV_GROUPS:
        # [p, c*B + n] = A[n, c*128 + p]: per-partition slab reads contiguous
        at_dram[key] = nc.dram_tensor(f"at_{key}", [P, dc * B], _FP8,
                                      kind="ExternalInput")
        w_dram[key] = nc.dram_tensor(f"w_{key}", [P, dc * FE], _FP16,
                                     kind="ExternalInput")
    n_out_rows = len(DEV_GROUPS) * FE
    out = nc.dram_tensor("out", [n_out_rows, B], _FP32, kind="ExternalOutput")

    with tile.TileContext(nc) as tc:
        queues = [nc.sync, nc.scalar, nc.gpsimd]
        qi = 0
        with (
            tc.tile_pool(name="wpool", bufs=1) as wpool,
            tc.tile_pool(name="apool", bufs=4) as apool,
            tc.tile_pool(name="opool", bufs=2) as opool,
            tc.tile_pool(name="accp", bufs=2, space="PSUM") as accp,
        ):
            # all weights resident in SBUF for the whole kernel (~23 KiB/par)
            w_sb = {}
            for key, _, _, _, dc in DEV_GROUPS:
                w = wpool.tile([P, dc, FE], _FP16, name=f"w_{key}")
                nc.sync.dma_start(
                    w, w_dram[key].rearrange("p (c f) -> p c f", f=FE))
                w_sb[key] = w

            for gi, (key, _, _, _, dc) in enumerate(DEV_GROUPS):
                acc = accp.tile([FE, B], _FP32, tag="acc", name=f"acc_{key}")
                for c0 in range(0, dc, SLAB):
                    ch = min(SLAB, dc - c0)
                    a_sb = apool.tile([P, SLAB, B], _FP8, tag="a")
                    queues[qi % 3].dma_start(
                        a_sb[:, :ch, :],
                        at_dram[key][:, c0 * B:(c0 + ch) * B]
                        .rearrange("p (c n) -> p c n", n=B))
                    qi += 1
                    for j in range(ch):
                        c = c0 + j
                        for nb in range(4):
                            nc.tensor.matmul(
                                acc[:, bass.ts(nb, 512)],
                                lhsT=w_sb[key][:, c, :],
                                rhs=a_sb[:, j, bass.ts(nb, 512)],
                                start=(c == 0),
                                stop=(c == dc - 1),
                            )
                acc_sb = opool.tile([FE, B], _FP32, tag="accsb")
                nc.vector.tensor_copy(acc_sb, acc)
                queues[qi % 3].dma_start(out[gi * FE:(gi + 1) * FE, :], acc_sb)
                qi += 1

    nc.finalize()
    return nc


_NC_CACHE: bass.Bass | None = None


def _get_nc() -> bass.Bass:
    global _NC_CACHE
    if _NC_CACHE is None:
        _NC_CACHE = _build()
    return _NC_CACHE


def _norm_rows(emb: np.ndarray, s: np.ndarray) -> np.ndarray:
    mask = s != 0
    safe = np.where(mask, s, 1.0).astype(np.float32)
    return np.where(mask[:, None], emb / safe[:, None], emb)


def kernel(**inputs: np.ndarray) -> np.ndarray:
    nc = _get_nc()

    # host repack: A^T as fp8 bit patterns (one strided transpose per group)
    at8 = {}
    for key, aname, _, v, dc in DEV_GROUPS:
        a = np.asarray(inputs[aname], dtype=np.int32)
        a8 = np.zeros((N_CORES * dc * P, B), np.uint8)
        np.multiply(a.T, _FP8_ONE, out=a8[:v], casting="unsafe")
        at8[key] = a8

    in_maps = []
    for core in range(N_CORES):
        m = {}
        for key, _, wname, v, dc in DEV_GROUPS:
            rows = dc * P
            sl = at8[key][core * rows:(core + 1) * rows]  # [rows, B]
            m[f"at_{key}"] = np.ascontiguousarray(
                sl.reshape(dc, P, B).transpose(1, 0, 2)
            ).reshape(P, dc * B).view(_NP_FP8)

            w = np.asarray(inputs[wname], dtype=np.float32)
            we = np.zeros((dc * P, FE), np.float16)
            v0 = core * rows
            n = max(0, min(v, v0 + rows) - v0)
            we[:n, :LF] = w[v0:v0 + n]
            we[:n, LF] = 1.0
            m[f"w_{key}"] = np.ascontiguousarray(
                we.reshape(dc, P, FE).transpose(1, 0, 2)).reshape(P, dc * FE)
        in_maps.append(m)

    trace = bool(int(os.environ.get("EMB_TRACE", "0")))
    res = run_bass_kernel_spmd(nc, in_maps, core_ids=list(range(N_CORES)),
                               trace=trace)
    if trace and res.exec_time_ns is not None:
        print(f"HW exec time: {res.exec_time_ns} ns")
        if res.instructions_and_trace is not None:
            print(f"trace: {res.instructions_and_trace[1]}")

    # host: sum vocab-slice partials across cores -> [B, FE] per group
    parts = {}
    for gi, (key, _, _, _, _) in enumerate(DEV_GROUPS):
        acc = np.zeros((FE, B), np.float32)
        for r in res.results:
            acc += r["out"][gi * FE:(gi + 1) * FE, :]
        parts[key] = (acc[:LF].T, acc[LF])  # [B, LF], row-sums [B]

    # host: the two tiny groups end to end
    for key, aname, wname, _ in HOST_GROUPS:
        a = np.asarray(inputs[aname], dtype=np.int32).astype(np.float32)
        w = np.asarray(inputs[wname], dtype=np.float32)
        parts[key] = (a @ w, a.sum(axis=1))

    # host: normalization quirks of the original module
    decades = _norm_rows(parts["dec"][0], parts["dec"][1])
    decades = _norm_rows(decades, parts["mov"][1])
    movies = parts["mov"][0]
    categories = _norm_rows(parts["cat"][0], parts["cat"][1])
    persons = _norm_rows(parts["per"][0], parts["per"][1])
    companies = _norm_rows(parts["com"][0], parts["com"][1])

    return np.concatenate(
        [decades, movies, categories, persons, companies], axis=1
    ).astype(np.float32)
